# revision 28
# baseline (speedup 1.0000x reference)
"""Multi-head attention (16 heads, d_model=1024, B=2, T=S=2048) on 8 trn2 cores.

Strategy: tensor-parallel over heads — 2 heads per core (head-dim slice of 128).
Per core, per batch b:
  - Q/K/V projections are emitted JUST-IN-TIME inside the first t-chunk's
    s-tile loop (K group j before st=4j, V pair before its first AV use, Q
    chunk at each chunk start), so the ~13MB of projection input DMA overlaps
    the attention pipeline instead of forming a serial lead-in phase.
  - scores^T[s,t] = K_h @ (Q_h/8)^T per head; the two heads' K=64 matmuls
    occupy disjoint PE row quadrants (partition bases 0/64) and run
    CONCURRENTLY on HW (verified in trace).
  - exp via ScalarE per head into a shared [128,2,512] SBUF tile; attn_bias
    enters multiplicatively (host sends exp(bias) bf16) via ONE fused
    [128,2x512] DVE multiply covering both heads.
  - ctx^T[d,t] = sum_s V[s,d]·P^T[s,t] accumulated over 16 s-tiles in PSUM,
    with an appended ones column giving the softmax denominator for free;
    split into two row-quadrant halves (K=64 pairs, concurrent).
  - normalization via reciprocal + gpsimd partition broadcast.
  - out-projection of chunk c is emitted 3 s-tiles INTO chunk c+1's loop so
    its PE/evac work fills gaps of the Act-bound steady state.
All DMA is host pre-tiled: every transfer is a single contiguous HBM block
with >=4KB per-partition runs (bias/V pairs of s-tiles per DMA; q/k 8KB).
Host: pre-tiles/casts inputs (bf16) + exp(bias), sums the 8 partial outputs,
adds bo + bv@Wo.T (bk dropped: softmax shift-invariant).
"""

import sys

sys.path.insert(0, "/opt/trn_rl_repo")

from contextlib import ExitStack

import ml_dtypes
import numpy as np

from concourse import bacc, mybir
from concourse.bass import ts
from concourse.bass_utils import run_bass_kernel_spmd
from concourse.tile import TileContext
from concourse.tile_rust import add_dep_helper

B, T, S, D, H, HD = 2, 2048, 2048, 1024, 16, 64
NCORES = 8
HPC = H // NCORES  # 2 heads per core
DPC = HPC * HD  # 128 head-dims per core
DCH = D // 128  # 8 dmodel chunks
NST = S // 128  # 16 s-tiles
NCH = T // 512  # 4 t-chunks of 512
BF = mybir.dt.bfloat16
F32 = mybir.dt.float32
EXP = mybir.ActivationFunctionType.Exp
COPY = mybir.ActivationFunctionType.Copy
ADD = mybir.AluOpType.add
MULT = mybir.AluOpType.mult

_PROGRAM = None


def build_program():
    nc = bacc.Bacc()
    qTt = nc.declare_dram_parameter("qTt", [B, NCH, 128, DCH, 512], BF, isOutput=False)
    kTt = nc.declare_dram_parameter("kTt", [B, NCH, 128, DCH, 512], BF, isOutput=False)
    vTt = nc.declare_dram_parameter(
        "vTt", [B, NST // 2, 128, 2, DCH, 128], BF, isOutput=False
    )
    biasTt = nc.declare_dram_parameter(
        "biasTt", [B, NCH, NST // 2, 128, 2, HPC, 512], BF, isOutput=False
    )
    wqt = nc.declare_dram_parameter("wqt", [128, DCH, DPC], BF, isOutput=False)
    wkt = nc.declare_dram_parameter("wkt", [128, DCH, DPC], BF, isOutput=False)
    wvt = nc.declare_dram_parameter("wvt", [128, DCH, DPC], BF, isOutput=False)
    woT = nc.declare_dram_parameter("woT", [DPC, D], BF, isOutput=False)
    bq_d = nc.declare_dram_parameter("bq", [DPC, 1], F32, isOutput=False)
    outp = nc.declare_dram_parameter("outp", [B, T, 2, 512], BF, isOutput=True)

    with TileContext(nc) as tc, ExitStack() as ctx:
        consts = ctx.enter_context(tc.tile_pool(name="consts", bufs=1))
        ldqk_pool = ctx.enter_context(tc.tile_pool(name="ldqk", bufs=3))
        ldv_pool = ctx.enter_context(tc.tile_pool(name="ldv", bufs=3))
        qk_pool = ctx.enter_context(tc.tile_pool(name="qk", bufs=1))
        qtc_pool = ctx.enter_context(tc.tile_pool(name="qtc", bufs=2))
        v_pool = ctx.enter_context(tc.tile_pool(name="vsb", bufs=1))
        bias_pool = ctx.enter_context(tc.tile_pool(name="bias", bufs=28))
        pt_pool = ctx.enter_context(tc.tile_pool(name="pt", bufs=6))
        norm_pool = ctx.enter_context(tc.tile_pool(name="norm", bufs=2))
        ctxt_pool = ctx.enter_context(tc.tile_pool(name="ctxt", bufs=2))
        outs_pool = ctx.enter_context(tc.tile_pool(name="outs", bufs=2))
        # PSUM: scores get two 2-bank slots (both heads side by side so ONE
        # fused activation exps both); projections/out-proj share two 1-bank
        # slots; ctx accumulators take the last 2 banks (one per head).
        sc_pool = ctx.enter_context(tc.tile_pool(name="sc", bufs=2, space="PSUM"))
        ps_pool = ctx.enter_context(tc.tile_pool(name="ps", bufs=2, space="PSUM"))
        ctx_ps_pool = ctx.enter_context(
            tc.tile_pool(name="ctx_ps", bufs=1, space="PSUM")
        )

        wq_sb = consts.tile([128, DCH, DPC], BF, tag="wq")
        wk_sb = consts.tile([128, DCH, DPC], BF, tag="wk")
        wv_sb = consts.tile([128, DCH, DPC], BF, tag="wv")
        wo_sb = consts.tile([DPC, D], BF, tag="wo")
        bq_sb = consts.tile([DPC, 1], F32, tag="bq")
        for w_sb, w_d in [(wq_sb, wqt), (wk_sb, wkt), (wv_sb, wvt)]:
            nc.sync.dma_start(out=w_sb, in_=w_d[:])
        nc.sync.dma_start(out=wo_sb, in_=woT[:])
        nc.sync.dma_start(out=bq_sb, in_=bq_d[:])

        # K / V stay resident per batch; double-buffered across the two
        # batches so batch 1's projections run during batch 0's chunks.
        KT_sbs = [qk_pool.tile([DPC, S], BF, tag=f"KT{p}", name=f"KT{p}") for p in range(2)]
        v_tiles = [
            [
                v_pool.tile([128, HPC, HD + 1], BF, tag=f"v{p}_{st}", name=f"v{p}_{st}")
                for st in range(NST)
            ]
            for p in range(2)
        ]
        # ones columns (softmax denominator trick) written once; projection
        # rewrites only cols 0:64 so these survive.
        for p in range(2):
            for st in range(NST):
                for h in range(HPC):
                    nc.vector.memset(v_tiles[p][st][:, h, HD : HD + 1], 1.0)

        pending = None  # (ctxT tile, b, chunk) awaiting out-projection

        def emit_out_proj_tt(pend, tt):
            ctxT, ob, och = pend
            out_sb = outs_pool.tile([128, 2, 512], BF, tag="out")
            for eh in range(2):
                po = ps_pool.tile([128, 512], F32, tag="ps", name=f"po{tt}{eh}")
                nc.tensor.matmul(
                    po[:],
                    lhsT=ctxT[:, ts(tt, 128)],
                    rhs=wo_sb[:, ts(eh, 512)],
                    start=True,
                    stop=True,
                )
                if eh == 0:
                    nc.vector.tensor_copy(out=out_sb[:, eh, :], in_=po[:])
                else:
                    nc.scalar.activation(out=out_sb[:, eh, :], in_=po[:], func=COPY)
            t0 = och * 512 + tt * 128
            nc.sync.dma_start(out=outp[ob, t0 : t0 + 128], in_=out_sb)

        def emit_out_proj(pend):
            for tt in range(4):
                emit_out_proj_tt(pend, tt)

        def emit_qproj(qb, qchk):
            qt_sb = ldqk_pool.tile([128, DCH, 512], BF, tag="ldqk")
            nc.sync.dma_start(out=qt_sb, in_=qTt[qb, qchk])
            pq = ps_pool.tile([128, 512], F32, tag="ps", name="pq")
            for c in range(DCH):
                nc.tensor.matmul(
                    pq[:],
                    lhsT=wq_sb[:, c, :],
                    rhs=qt_sb[:, c, :],
                    start=(c == 0),
                    stop=(c == DCH - 1),
                )
            QTc = qtc_pool.tile([DPC, 512], BF, tag="QTc")
            # QT = Q/8 + bq/8 (host sends bq/8; attention scale folded
            # in). On Act so scores never queue behind DVE work.
            nc.scalar.activation(
                out=QTc[:],
                in_=pq[:],
                func=mybir.ActivationFunctionType.Identity,
                bias=bq_sb[:],
                scale=0.125,
            )
            return QTc

        def emit_kgroup(pb, j):
            kt_sb = ldqk_pool.tile([128, DCH, 512], BF, tag="ldqk")
            nc.sync.dma_start(out=kt_sb, in_=kTt[pb, j])
            pk = ps_pool.tile([128, 512], F32, tag="ps", name="pk")
            for c in range(DCH):
                nc.tensor.matmul(
                    pk[:],
                    lhsT=wk_sb[:, c, :],
                    rhs=kt_sb[:, c, :],
                    start=(c == 0),
                    stop=(c == DCH - 1),
                )
            nc.vector.tensor_copy(out=KT_sbs[pb % 2][:, ts(j, 512)], in_=pk[:])

        def emit_vpair(pb, p):
            vt_sb = ldv_pool.tile([128, 2, DCH, 128], BF, tag="ldv")
            nc.sync.dma_start(out=vt_sb, in_=vTt[pb, p])
            for k in range(2):
                pv = ps_pool.tile([128, 512], F32, tag="ps", name="pv")
                for c in range(DCH):
                    nc.tensor.matmul(
                        pv[:, 0:DPC],
                        lhsT=vt_sb[:, k, c, :],
                        rhs=wv_sb[:, c, :],
                        start=(c == 0),
                        stop=(c == DCH - 1),
                    )
                for h in range(HPC):
                    nc.vector.tensor_copy(
                        out=v_tiles[pb % 2][2 * p + k][:, h, 0:HD],
                        in_=pv[:, ts(h, HD)],
                    )

        chunk_seq = [(b, chk) for b in range(B) for chk in range(NCH)]
        # K/V projection schedule: each batch self-JITs inside its first
        # chunk — the K group / V pair task is emitted just before the
        # s-tiles that consume it, so projection DMA+compute overlap the
        # attention pipeline instead of forming a serial lead-in.
        side = {ci: {} for ci in range(len(chunk_seq))}
        for pb in range(B):
            ci0 = pb * NCH
            for j in range(4):
                side[ci0].setdefault(4 * j, []).append((emit_kgroup, pb, j))
            for p in range(8):
                side[ci0].setdefault(2 * p, []).append((emit_vpair, pb, p))

        QTc_next = None
        for ci, (b, chk) in enumerate(chunk_seq):
            if True:
                # Q projection: prefetched at st==8 of the previous chunk so
                # the first scores of this chunk start right at the boundary.
                QTc = QTc_next if QTc_next is not None else emit_qproj(b, chk)
                QTc_next = None
                KT_sb = KT_sbs[b % 2]

                cps = [
                    ctx_ps_pool.tile([128, 512], F32, tag=f"ctx{h}", name=f"cps{h}")
                    for h in range(HPC)
                ]
                for st in range(NST):
                    for task in side[ci].get(st, []):
                        task[0](*task[1:])

                    if st % 2 == 0:
                        bias_sb = bias_pool.tile([128, 2, HPC, 512], BF, tag="bias")
                        nc.sync.dma_start(out=bias_sb, in_=biasTt[b, chk, st // 2])

                    sch = sc_pool.tile([128, HPC, 512], F32, tag="sc", name="sc")
                    sc_mms = []
                    with tc.high_priority(offset=400):
                        for h in range(HPC):
                            mm = nc.tensor.matmul(
                                sch[:, h, :],
                                lhsT=KT_sb[ts(h, HD), ts(st, 128)],
                                rhs=QTc[ts(h, HD), :],
                                start=True,
                                stop=True,
                            )
                            sc_mms.append(mm)
                    add_dep_helper(
                        sc_mms[1].ins, sc_mms[0].ins, sync=False,
                        reason="score pair adjacency",
                    )
                    pt = pt_pool.tile([128, HPC, 512], BF, tag="pt")
                    # one fused exp over both heads' scores (2-bank PSUM read)
                    nc.scalar.activation(out=pt[:], in_=sch[:], func=EXP)
                    # attn_bias enters multiplicatively: host sends exp(bias);
                    # one fused bf16 multiply covers both heads. (GpSimd
                    # measured 2.1us for this op vs DVE 0.65us — keep on DVE.)
                    nc.vector.tensor_tensor(
                        out=pt[:], in0=pt[:], in1=bias_sb[:, st % 2], op=MULT
                    )
                    for h in range(HPC):
                        nc.tensor.matmul(
                            cps[h][0 : HD + 1, :],
                            lhsT=v_tiles[b % 2][st][:, h, :],
                            rhs=pt[:, h, :],
                            start=(st == 0),
                            stop=(st == NST - 1),
                        )
                    if st == 2 and pending is not None:
                        emit_out_proj(pending)
                        pending = None
                    if st == 8 and ci + 1 < len(chunk_seq):
                        QTc_next = emit_qproj(*chunk_seq[ci + 1])

                # ---- chunk boundary: normalize into ctxT ----
                # ctx and its denominator (row 64, from the ones column) sit
                # in one PSUM bank per head; normalization reads PSUM once.
                ctxT = ctxt_pool.tile([DPC, 512], BF, tag="ctxT")
                for h in range(HPC):
                    # reciprocal_approx_fast needs base partition 0
                    den0 = norm_pool.tile([1, 512], F32, tag="den0", name=f"den{h}")
                    nc.scalar.activation(
                        out=den0[:], in_=cps[h][HD : HD + 1, :], func=COPY
                    )
                    rd = norm_pool.tile([1, 512], F32, tag="rd")
                    nc.vector.reciprocal_approx_fast(out=rd[:], in_=den0[:])
                    rrep = norm_pool.tile([128, 512], F32, tag="rrep")
                    nc.gpsimd.partition_broadcast(rrep[:], rd[:])
                    nc.vector.tensor_tensor(
                        out=ctxT[ts(h, HD), :],
                        in0=cps[h][0:HD, :],
                        in1=rrep[0:HD, :],
                        op=MULT,
                    )
                pending = (ctxT, b, chk)

        emit_out_proj(pending)

    nc.compile()
    return nc


def _get_program():
    global _PROGRAM
    if _PROGRAM is None:
        _PROGRAM = build_program()
    return _PROGRAM


def make_in_maps(query, key, value, attn_bias, Wq, bq, Wk, Wv, Wo):
    bf = ml_dtypes.bfloat16
    f32 = np.float32
    query = np.asarray(query, f32)
    key = np.asarray(key, f32)
    value = np.asarray(value, f32)
    attn_bias = np.asarray(attn_bias, f32)
    Wq, Wk, Wv, Wo = (np.asarray(w, f32) for w in (Wq, Wk, Wv, Wo))
    # q/k: [b, ch, p, c, t']; v: [b, st2, p, s2, c, s'] — per-DMA contiguous
    qTt = np.ascontiguousarray(
        query.reshape(B, NCH, 512, DCH, 128).transpose(0, 1, 4, 3, 2)
    ).astype(bf)
    kTt = np.ascontiguousarray(
        key.reshape(B, NCH, 512, DCH, 128).transpose(0, 1, 4, 3, 2)
    ).astype(bf)
    vTt = np.ascontiguousarray(
        value.reshape(B, NST // 2, 2, 128, DCH, 128).transpose(0, 1, 5, 2, 4, 3)
    ).astype(bf)
    in_maps = []
    for cidx in range(NCORES):
        dsl = slice(DPC * cidx, DPC * (cidx + 1))
        hsl = slice(HPC * cidx, HPC * (cidx + 1))
        # [b, h, ch, t', st2, s2, p] -> [b, ch, st2, p, s2, h, t']
        biasTt = np.ascontiguousarray(
            np.exp(attn_bias[:, hsl])
            .reshape(B, HPC, NCH, 512, NST // 2, 2, 128)
            .transpose(0, 2, 4, 6, 5, 1, 3)
        ).astype(bf)
        in_maps.append(
            {
                "qTt": qTt,
                "kTt": kTt,
                "vTt": vTt,
                "biasTt": biasTt,
                "wqt": np.ascontiguousarray(
                    Wq[dsl].T.reshape(DCH, 128, DPC).transpose(1, 0, 2)
                ).astype(bf),
                "wkt": np.ascontiguousarray(
                    Wk[dsl].T.reshape(DCH, 128, DPC).transpose(1, 0, 2)
                ).astype(bf),
                "wvt": np.ascontiguousarray(
                    Wv[dsl].T.reshape(DCH, 128, DPC).transpose(1, 0, 2)
                ).astype(bf),
                "woT": np.ascontiguousarray(Wo[:, dsl].T).astype(bf),
                # bq/8: the Q-projection evacuation computes Q*0.125 + this
                "bq": np.ascontiguousarray(
                    np.asarray(bq, f32)[dsl] * 0.125
                ).reshape(DPC, 1),
            }
        )
    return in_maps


def combine_outputs(results, Wo, bv, bo):
    out = np.zeros((B, T, D), np.float64)
    for c in range(NCORES):
        out += results[c]["outp"].reshape(B, T, D).astype(np.float64)
    const = np.asarray(bv, np.float64) @ np.asarray(Wo, np.float64).T + np.asarray(
        bo, np.float64
    )
    out += const
    return out.astype(np.float32)


def kernel(
    query,
    key,
    value,
    attn_bias,
    key_padding_mask,
    Wq,
    bq,
    Wk,
    bk,
    Wv,
    bv,
    Wo,
    bo,
):
    # key_padding_mask is all-False in this problem; bk is dropped (softmax is
    # invariant to a per-row constant shift); bv/bo enter via a host constant.
    nc = _get_program()
    in_maps = make_in_maps(query, key, value, attn_bias, Wq, bq, Wk, Wv, Wo)
    res = run_bass_kernel_spmd(nc, in_maps, list(range(NCORES)))
    return combine_outputs(res.results, Wo, bv, bo)


if __name__ == "__main__":
    rng = np.random.default_rng(0)
    args = {
        "query": rng.standard_normal((B, T, D), np.float32),
        "key": rng.standard_normal((B, S, D), np.float32),
        "value": rng.standard_normal((B, S, D), np.float32),
        "attn_bias": rng.standard_normal((B, H, T, S), np.float32),
        "key_padding_mask": np.zeros((B, S), bool),
        "Wq": rng.uniform(-0.03125, 0.03125, (D, D)).astype(np.float32),
        "bq": rng.uniform(-0.03125, 0.03125, D).astype(np.float32),
        "Wk": rng.uniform(-0.03125, 0.03125, (D, D)).astype(np.float32),
        "bk": rng.uniform(-0.03125, 0.03125, D).astype(np.float32),
        "Wv": rng.uniform(-0.03125, 0.03125, (D, D)).astype(np.float32),
        "bv": rng.uniform(-0.03125, 0.03125, D).astype(np.float32),
        "Wo": rng.uniform(-0.03125, 0.03125, (D, D)).astype(np.float32),
        "bo": rng.uniform(-0.03125, 0.03125, D).astype(np.float32),
    }
    out = kernel(**args)
    print("kernel ran, out shape", out.shape, "std", out.std())


# revision 29
# speedup vs baseline: 1.0201x; 1.0201x over previous
"""Multi-head attention (16 heads, d_model=1024, B=2, T=S=2048) on 8 trn2 cores.

Strategy: tensor-parallel over heads — 2 heads per core (head-dim slice of 128).
Per core, per batch b:
  - Q/K/V projections are emitted JUST-IN-TIME inside the first t-chunk's
    s-tile loop (K group j before st=4j, V pair before its first AV use, Q
    chunk at each chunk start), so the ~13MB of projection input DMA overlaps
    the attention pipeline instead of forming a serial lead-in phase.
  - scores^T[s,t] = K_h @ (Q_h/8)^T per head; the two heads' K=64 matmuls
    occupy disjoint PE row quadrants (partition bases 0/64) and run
    CONCURRENTLY on HW (verified in trace).
  - exp via ScalarE per head into a shared [128,2,512] SBUF tile; attn_bias
    enters multiplicatively (host sends exp(bias) bf16) via ONE fused
    [128,2x512] DVE multiply covering both heads.
  - ctx^T[d,t] = sum_s V[s,d]·P^T[s,t] accumulated over 16 s-tiles in PSUM,
    with an appended ones column giving the softmax denominator for free;
    split into two row-quadrant halves (K=64 pairs, concurrent).
  - normalization via reciprocal + gpsimd partition broadcast.
  - out-projection of chunk c is emitted 3 s-tiles INTO chunk c+1's loop so
    its PE/evac work fills gaps of the Act-bound steady state.
All DMA is host pre-tiled: every transfer is a single contiguous HBM block
with >=4KB per-partition runs (bias/V pairs of s-tiles per DMA; q/k 8KB).
Host: pre-tiles/casts inputs (bf16) + exp(bias), sums the 8 partial outputs,
adds bo + bv@Wo.T (bk dropped: softmax shift-invariant).
"""

import sys

sys.path.insert(0, "/opt/trn_rl_repo")

from contextlib import ExitStack

import ml_dtypes
import numpy as np

from concourse import bacc, mybir
from concourse.bass import ts
from concourse.bass_utils import run_bass_kernel_spmd
from concourse.tile import TileContext
from concourse.tile_rust import add_dep_helper

B, T, S, D, H, HD = 2, 2048, 2048, 1024, 16, 64
NCORES = 8
HPC = H // NCORES  # 2 heads per core
DPC = HPC * HD  # 128 head-dims per core
DCH = D // 128  # 8 dmodel chunks
NST = S // 128  # 16 s-tiles
NCH = T // 512  # 4 t-chunks of 512
BF = mybir.dt.bfloat16
F32 = mybir.dt.float32
EXP = mybir.ActivationFunctionType.Exp
COPY = mybir.ActivationFunctionType.Copy
ADD = mybir.AluOpType.add
MULT = mybir.AluOpType.mult

_PROGRAM = None


def build_program():
    nc = bacc.Bacc()
    qTt = nc.declare_dram_parameter("qTt", [B, NCH, 128, DCH, 512], BF, isOutput=False)
    kTt = nc.declare_dram_parameter("kTt", [B, NCH, 128, DCH, 512], BF, isOutput=False)
    vTt = nc.declare_dram_parameter(
        "vTt", [B, NST // 2, 128, 2, DCH, 128], BF, isOutput=False
    )
    biasTt = nc.declare_dram_parameter(
        "biasTt", [B, NCH, NST // 2, 128, 2, HPC, 512], BF, isOutput=False
    )
    wqt = nc.declare_dram_parameter("wqt", [128, DCH, DPC], BF, isOutput=False)
    wkt = nc.declare_dram_parameter("wkt", [128, DCH, DPC], BF, isOutput=False)
    wvt = nc.declare_dram_parameter("wvt", [128, DCH, DPC], BF, isOutput=False)
    woT = nc.declare_dram_parameter("woT", [DPC, D], BF, isOutput=False)
    bq_d = nc.declare_dram_parameter("bq", [DPC, 1], F32, isOutput=False)
    outp = nc.declare_dram_parameter("outp", [B, T, 2, 512], BF, isOutput=True)

    with TileContext(nc) as tc, ExitStack() as ctx:
        consts = ctx.enter_context(tc.tile_pool(name="consts", bufs=1))
        ldqk_pool = ctx.enter_context(tc.tile_pool(name="ldqk", bufs=3))
        ldv_pool = ctx.enter_context(tc.tile_pool(name="ldv", bufs=3))
        qk_pool = ctx.enter_context(tc.tile_pool(name="qk", bufs=1))
        qtc_pool = ctx.enter_context(tc.tile_pool(name="qtc", bufs=2))
        v_pool = ctx.enter_context(tc.tile_pool(name="vsb", bufs=1))
        bias_pool = ctx.enter_context(tc.tile_pool(name="bias", bufs=26))
        pt_pool = ctx.enter_context(tc.tile_pool(name="pt", bufs=8))
        norm_pool = ctx.enter_context(tc.tile_pool(name="norm", bufs=2))
        ctxt_pool = ctx.enter_context(tc.tile_pool(name="ctxt", bufs=2))
        outs_pool = ctx.enter_context(tc.tile_pool(name="outs", bufs=3))
        # PSUM: scores get two 2-bank slots (both heads side by side so ONE
        # fused activation exps both); projections/out-proj share two 1-bank
        # slots; ctx accumulators take the last 2 banks (one per head).
        sc_pool = ctx.enter_context(tc.tile_pool(name="sc", bufs=2, space="PSUM"))
        ps_pool = ctx.enter_context(tc.tile_pool(name="ps", bufs=2, space="PSUM"))
        ctx_ps_pool = ctx.enter_context(
            tc.tile_pool(name="ctx_ps", bufs=1, space="PSUM")
        )

        wq_sb = consts.tile([128, DCH, DPC], BF, tag="wq")
        wk_sb = consts.tile([128, DCH, DPC], BF, tag="wk")
        wv_sb = consts.tile([128, DCH, DPC], BF, tag="wv")
        wo_sb = consts.tile([DPC, D], BF, tag="wo")
        bq_sb = consts.tile([DPC, 1], F32, tag="bq")
        for w_sb, w_d in [(wq_sb, wqt), (wk_sb, wkt), (wv_sb, wvt)]:
            nc.sync.dma_start(out=w_sb, in_=w_d[:])
        nc.sync.dma_start(out=wo_sb, in_=woT[:])
        nc.sync.dma_start(out=bq_sb, in_=bq_d[:])

        # K / V stay resident per batch; double-buffered across the two
        # batches so batch 1's projections run during batch 0's chunks.
        KT_sbs = [qk_pool.tile([DPC, S], BF, tag=f"KT{p}", name=f"KT{p}") for p in range(2)]
        v_tiles = [
            [
                v_pool.tile([128, HPC, HD + 1], BF, tag=f"v{p}_{st}", name=f"v{p}_{st}")
                for st in range(NST)
            ]
            for p in range(2)
        ]
        # ones columns (softmax denominator trick) written once; projection
        # rewrites only cols 0:64 so these survive.
        for p in range(2):
            for st in range(NST):
                for h in range(HPC):
                    nc.vector.memset(v_tiles[p][st][:, h, HD : HD + 1], 1.0)

        pending = None  # (ctxT tile, b, chunk) awaiting out-projection

        def emit_out_proj_tt(pend, tt):
            ctxT, ob, och = pend
            out_sb = outs_pool.tile([128, 2, 512], BF, tag="out")
            for eh in range(2):
                po = ps_pool.tile([128, 512], F32, tag="ps", name=f"po{tt}{eh}")
                nc.tensor.matmul(
                    po[:],
                    lhsT=ctxT[:, ts(tt, 128)],
                    rhs=wo_sb[:, ts(eh, 512)],
                    start=True,
                    stop=True,
                )
                if eh == 0:
                    nc.vector.tensor_copy(out=out_sb[:, eh, :], in_=po[:])
                else:
                    nc.scalar.activation(out=out_sb[:, eh, :], in_=po[:], func=COPY)
            t0 = och * 512 + tt * 128
            nc.sync.dma_start(out=outp[ob, t0 : t0 + 128], in_=out_sb)

        def emit_out_proj(pend):
            for tt in range(4):
                emit_out_proj_tt(pend, tt)

        def emit_qproj(qb, qchk):
            qt_sb = ldqk_pool.tile([128, DCH, 512], BF, tag="ldqk")
            nc.sync.dma_start(out=qt_sb, in_=qTt[qb, qchk])
            pq = ps_pool.tile([128, 512], F32, tag="ps", name="pq")
            for c in range(DCH):
                nc.tensor.matmul(
                    pq[:],
                    lhsT=wq_sb[:, c, :],
                    rhs=qt_sb[:, c, :],
                    start=(c == 0),
                    stop=(c == DCH - 1),
                )
            QTc = qtc_pool.tile([DPC, 512], BF, tag="QTc")
            # QT = Q/8 + bq/8 (host sends bq/8; attention scale folded
            # in). On Act so scores never queue behind DVE work.
            nc.scalar.activation(
                out=QTc[:],
                in_=pq[:],
                func=mybir.ActivationFunctionType.Identity,
                bias=bq_sb[:],
                scale=0.125,
            )
            return QTc

        def emit_kgroup(pb, j):
            kt_sb = ldqk_pool.tile([128, DCH, 512], BF, tag="ldqk")
            nc.sync.dma_start(out=kt_sb, in_=kTt[pb, j])
            pk = ps_pool.tile([128, 512], F32, tag="ps", name="pk")
            for c in range(DCH):
                nc.tensor.matmul(
                    pk[:],
                    lhsT=wk_sb[:, c, :],
                    rhs=kt_sb[:, c, :],
                    start=(c == 0),
                    stop=(c == DCH - 1),
                )
            nc.vector.tensor_copy(out=KT_sbs[pb % 2][:, ts(j, 512)], in_=pk[:])

        def emit_vpair(pb, p):
            vt_sb = ldv_pool.tile([128, 2, DCH, 128], BF, tag="ldv")
            nc.sync.dma_start(out=vt_sb, in_=vTt[pb, p])
            for k in range(2):
                pv = ps_pool.tile([128, 512], F32, tag="ps", name="pv")
                for c in range(DCH):
                    nc.tensor.matmul(
                        pv[:, 0:DPC],
                        lhsT=vt_sb[:, k, c, :],
                        rhs=wv_sb[:, c, :],
                        start=(c == 0),
                        stop=(c == DCH - 1),
                    )
                for h in range(HPC):
                    nc.vector.tensor_copy(
                        out=v_tiles[pb % 2][2 * p + k][:, h, 0:HD],
                        in_=pv[:, ts(h, HD)],
                    )

        chunk_seq = [(b, chk) for b in range(B) for chk in range(NCH)]
        # K/V projection schedule: each batch self-JITs inside its first
        # chunk — the K group / V pair task is emitted just before the
        # s-tiles that consume it, so projection DMA+compute overlap the
        # attention pipeline instead of forming a serial lead-in.
        side = {ci: {} for ci in range(len(chunk_seq))}
        for pb in range(B):
            ci0 = pb * NCH
            for j in range(4):
                side[ci0].setdefault(4 * j, []).append((emit_kgroup, pb, j))
            for p in range(8):
                side[ci0].setdefault(2 * p, []).append((emit_vpair, pb, p))

        QTc_next = None
        for ci, (b, chk) in enumerate(chunk_seq):
            if True:
                # Q projection: prefetched at st==8 of the previous chunk so
                # the first scores of this chunk start right at the boundary.
                QTc = QTc_next if QTc_next is not None else emit_qproj(b, chk)
                QTc_next = None
                KT_sb = KT_sbs[b % 2]

                cps = [
                    ctx_ps_pool.tile([128, 512], F32, tag=f"ctx{h}", name=f"cps{h}")
                    for h in range(HPC)
                ]
                for st in range(NST):
                    for task in side[ci].get(st, []):
                        task[0](*task[1:])

                    if st % 2 == 0:
                        bias_sb = bias_pool.tile([128, 2, HPC, 512], BF, tag="bias")
                        nc.sync.dma_start(out=bias_sb, in_=biasTt[b, chk, st // 2])

                    sch = sc_pool.tile([128, HPC, 512], F32, tag="sc", name="sc")
                    sc_mms = []
                    with tc.high_priority(offset=400):
                        for h in range(HPC):
                            mm = nc.tensor.matmul(
                                sch[:, h, :],
                                lhsT=KT_sb[ts(h, HD), ts(st, 128)],
                                rhs=QTc[ts(h, HD), :],
                                start=True,
                                stop=True,
                            )
                            sc_mms.append(mm)
                    add_dep_helper(
                        sc_mms[1].ins, sc_mms[0].ins, sync=False,
                        reason="score pair adjacency",
                    )
                    pt = pt_pool.tile([128, HPC, 512], BF, tag="pt")
                    # one fused exp over both heads' scores (2-bank PSUM read)
                    nc.scalar.activation(out=pt[:], in_=sch[:], func=EXP)
                    # attn_bias enters multiplicatively: host sends exp(bias);
                    # one fused bf16 multiply covers both heads. (GpSimd
                    # measured 2.1us for this op vs DVE 0.65us — keep on DVE.)
                    nc.vector.tensor_tensor(
                        out=pt[:], in0=pt[:], in1=bias_sb[:, st % 2], op=MULT
                    )
                    for h in range(HPC):
                        nc.tensor.matmul(
                            cps[h][0 : HD + 1, :],
                            lhsT=v_tiles[b % 2][st][:, h, :],
                            rhs=pt[:, h, :],
                            start=(st == 0),
                            stop=(st == NST - 1),
                        )
                    if st == 2 and pending is not None:
                        emit_out_proj(pending)
                        pending = None
                    if st == 8 and ci + 1 < len(chunk_seq):
                        QTc_next = emit_qproj(*chunk_seq[ci + 1])

                # ---- chunk boundary: normalize into ctxT ----
                # ctx and its denominator (row 64, from the ones column) sit
                # in one PSUM bank per head; normalization reads PSUM once.
                ctxT = ctxt_pool.tile([DPC, 512], BF, tag="ctxT")
                for h in range(HPC):
                    # reciprocal_approx_fast needs base partition 0
                    den0 = norm_pool.tile([1, 512], F32, tag="den0", name=f"den{h}")
                    nc.scalar.activation(
                        out=den0[:], in_=cps[h][HD : HD + 1, :], func=COPY
                    )
                    rd = norm_pool.tile([1, 512], F32, tag="rd")
                    nc.vector.reciprocal_approx_fast(out=rd[:], in_=den0[:])
                    rrep = norm_pool.tile([128, 512], F32, tag="rrep")
                    nc.gpsimd.partition_broadcast(rrep[:], rd[:])
                    nc.vector.tensor_tensor(
                        out=ctxT[ts(h, HD), :],
                        in0=cps[h][0:HD, :],
                        in1=rrep[0:HD, :],
                        op=MULT,
                    )
                pending = (ctxT, b, chk)

        emit_out_proj(pending)

    nc.compile()
    return nc


def _get_program():
    global _PROGRAM
    if _PROGRAM is None:
        _PROGRAM = build_program()
    return _PROGRAM


def make_in_maps(query, key, value, attn_bias, Wq, bq, Wk, Wv, Wo):
    bf = ml_dtypes.bfloat16
    f32 = np.float32
    query = np.asarray(query, f32)
    key = np.asarray(key, f32)
    value = np.asarray(value, f32)
    attn_bias = np.asarray(attn_bias, f32)
    Wq, Wk, Wv, Wo = (np.asarray(w, f32) for w in (Wq, Wk, Wv, Wo))
    # q/k: [b, ch, p, c, t']; v: [b, st2, p, s2, c, s'] — per-DMA contiguous
    qTt = np.ascontiguousarray(
        query.reshape(B, NCH, 512, DCH, 128).transpose(0, 1, 4, 3, 2)
    ).astype(bf)
    kTt = np.ascontiguousarray(
        key.reshape(B, NCH, 512, DCH, 128).transpose(0, 1, 4, 3, 2)
    ).astype(bf)
    vTt = np.ascontiguousarray(
        value.reshape(B, NST // 2, 2, 128, DCH, 128).transpose(0, 1, 5, 2, 4, 3)
    ).astype(bf)
    in_maps = []
    for cidx in range(NCORES):
        dsl = slice(DPC * cidx, DPC * (cidx + 1))
        hsl = slice(HPC * cidx, HPC * (cidx + 1))
        # [b, h, ch, t', st2, s2, p] -> [b, ch, st2, p, s2, h, t']
        biasTt = np.ascontiguousarray(
            np.exp(attn_bias[:, hsl])
            .reshape(B, HPC, NCH, 512, NST // 2, 2, 128)
            .transpose(0, 2, 4, 6, 5, 1, 3)
        ).astype(bf)
        in_maps.append(
            {
                "qTt": qTt,
                "kTt": kTt,
                "vTt": vTt,
                "biasTt": biasTt,
                "wqt": np.ascontiguousarray(
                    Wq[dsl].T.reshape(DCH, 128, DPC).transpose(1, 0, 2)
                ).astype(bf),
                "wkt": np.ascontiguousarray(
                    Wk[dsl].T.reshape(DCH, 128, DPC).transpose(1, 0, 2)
                ).astype(bf),
                "wvt": np.ascontiguousarray(
                    Wv[dsl].T.reshape(DCH, 128, DPC).transpose(1, 0, 2)
                ).astype(bf),
                "woT": np.ascontiguousarray(Wo[:, dsl].T).astype(bf),
                # bq/8: the Q-projection evacuation computes Q*0.125 + this
                "bq": np.ascontiguousarray(
                    np.asarray(bq, f32)[dsl] * 0.125
                ).reshape(DPC, 1),
            }
        )
    return in_maps


def combine_outputs(results, Wo, bv, bo):
    out = np.zeros((B, T, D), np.float64)
    for c in range(NCORES):
        out += results[c]["outp"].reshape(B, T, D).astype(np.float64)
    const = np.asarray(bv, np.float64) @ np.asarray(Wo, np.float64).T + np.asarray(
        bo, np.float64
    )
    out += const
    return out.astype(np.float32)


def kernel(
    query,
    key,
    value,
    attn_bias,
    key_padding_mask,
    Wq,
    bq,
    Wk,
    bk,
    Wv,
    bv,
    Wo,
    bo,
):
    # key_padding_mask is all-False in this problem; bk is dropped (softmax is
    # invariant to a per-row constant shift); bv/bo enter via a host constant.
    nc = _get_program()
    in_maps = make_in_maps(query, key, value, attn_bias, Wq, bq, Wk, Wv, Wo)
    res = run_bass_kernel_spmd(nc, in_maps, list(range(NCORES)))
    return combine_outputs(res.results, Wo, bv, bo)


if __name__ == "__main__":
    rng = np.random.default_rng(0)
    args = {
        "query": rng.standard_normal((B, T, D), np.float32),
        "key": rng.standard_normal((B, S, D), np.float32),
        "value": rng.standard_normal((B, S, D), np.float32),
        "attn_bias": rng.standard_normal((B, H, T, S), np.float32),
        "key_padding_mask": np.zeros((B, S), bool),
        "Wq": rng.uniform(-0.03125, 0.03125, (D, D)).astype(np.float32),
        "bq": rng.uniform(-0.03125, 0.03125, D).astype(np.float32),
        "Wk": rng.uniform(-0.03125, 0.03125, (D, D)).astype(np.float32),
        "bk": rng.uniform(-0.03125, 0.03125, D).astype(np.float32),
        "Wv": rng.uniform(-0.03125, 0.03125, (D, D)).astype(np.float32),
        "bv": rng.uniform(-0.03125, 0.03125, D).astype(np.float32),
        "Wo": rng.uniform(-0.03125, 0.03125, (D, D)).astype(np.float32),
        "bo": rng.uniform(-0.03125, 0.03125, D).astype(np.float32),
    }
    out = kernel(**args)
    print("kernel ran, out shape", out.shape, "std", out.std())


# revision 32
# speedup vs baseline: 1.0321x; 1.0118x over previous
"""Multi-head attention (16 heads, d_model=1024, B=2, T=S=2048) on 8 trn2 cores.

Strategy: tensor-parallel over heads — 2 heads per core (head-dim slice of 128).
Per core, per batch b:
  - Q/K/V projections are emitted JUST-IN-TIME inside the first t-chunk's
    s-tile loop (K group j before st=4j, V pair before its first AV use, Q
    chunk at each chunk start), so the ~13MB of projection input DMA overlaps
    the attention pipeline instead of forming a serial lead-in phase.
  - scores^T[s,t] = K_h @ (Q_h/8)^T per head; the two heads' K=64 matmuls
    occupy disjoint PE row quadrants (partition bases 0/64) and run
    CONCURRENTLY on HW (verified in trace).
  - exp via ScalarE per head into a shared [128,2,512] SBUF tile; attn_bias
    enters multiplicatively (host sends exp(bias) bf16) via ONE fused
    [128,2x512] DVE multiply covering both heads.
  - ctx^T[d,t] = sum_s V[s,d]·P^T[s,t] accumulated over 16 s-tiles in PSUM,
    with an appended ones column giving the softmax denominator for free;
    split into two row-quadrant halves (K=64 pairs, concurrent).
  - normalization via reciprocal + gpsimd partition broadcast.
  - out-projection of chunk c is emitted 3 s-tiles INTO chunk c+1's loop so
    its PE/evac work fills gaps of the Act-bound steady state.
All DMA is host pre-tiled: every transfer is a single contiguous HBM block
with >=4KB per-partition runs (bias/V pairs of s-tiles per DMA; q/k 8KB).
Host: pre-tiles/casts inputs (bf16) + exp(bias), sums the 8 partial outputs,
adds bo + bv@Wo.T (bk dropped: softmax shift-invariant).
"""

import sys

sys.path.insert(0, "/opt/trn_rl_repo")

from contextlib import ExitStack

import ml_dtypes
import numpy as np

from concourse import bacc, mybir
from concourse.bass import ts
from concourse.bass_utils import run_bass_kernel_spmd
from concourse.tile import TileContext
from concourse.tile_rust import add_dep_helper

B, T, S, D, H, HD = 2, 2048, 2048, 1024, 16, 64
NCORES = 8
HPC = H // NCORES  # 2 heads per core
DPC = HPC * HD  # 128 head-dims per core
DCH = D // 128  # 8 dmodel chunks
NST = S // 128  # 16 s-tiles
NCH = T // 512  # 4 t-chunks of 512
BF = mybir.dt.bfloat16
F32 = mybir.dt.float32
EXP = mybir.ActivationFunctionType.Exp
COPY = mybir.ActivationFunctionType.Copy
ADD = mybir.AluOpType.add
MULT = mybir.AluOpType.mult

_PROGRAM = None


def build_program():
    nc = bacc.Bacc()
    qTt = nc.declare_dram_parameter("qTt", [B, NCH, 128, DCH, 512], BF, isOutput=False)
    kTt = nc.declare_dram_parameter("kTt", [B, NCH, 128, DCH, 512], BF, isOutput=False)
    vTt = nc.declare_dram_parameter(
        "vTt", [B, NST // 2, 128, 2, DCH, 128], BF, isOutput=False
    )
    biasTt = nc.declare_dram_parameter(
        "biasTt", [B, NCH, NST // 2, 128, 2, HPC, 512], BF, isOutput=False
    )
    wqt = nc.declare_dram_parameter("wqt", [128, DCH, DPC], BF, isOutput=False)
    wkt = nc.declare_dram_parameter("wkt", [128, DCH, DPC], BF, isOutput=False)
    wvt = nc.declare_dram_parameter("wvt", [128, DCH, DPC], BF, isOutput=False)
    woT = nc.declare_dram_parameter("woT", [DPC, D], BF, isOutput=False)
    bq_d = nc.declare_dram_parameter("bq", [DPC, 1], F32, isOutput=False)
    outp = nc.declare_dram_parameter("outp", [B, T, 2, 512], BF, isOutput=True)

    with TileContext(nc) as tc, ExitStack() as ctx:
        consts = ctx.enter_context(tc.tile_pool(name="consts", bufs=1))
        ldqk_pool = ctx.enter_context(tc.tile_pool(name="ldqk", bufs=3))
        ldv_pool = ctx.enter_context(tc.tile_pool(name="ldv", bufs=3))
        qk_pool = ctx.enter_context(tc.tile_pool(name="qk", bufs=1))
        qtc_pool = ctx.enter_context(tc.tile_pool(name="qtc", bufs=2))
        v_pool = ctx.enter_context(tc.tile_pool(name="vsb", bufs=1))
        bias_pool = ctx.enter_context(tc.tile_pool(name="bias", bufs=26))
        pt_pool = ctx.enter_context(tc.tile_pool(name="pt", bufs=8))
        norm_pool = ctx.enter_context(tc.tile_pool(name="norm", bufs=2))
        ctxt_pool = ctx.enter_context(tc.tile_pool(name="ctxt", bufs=2))
        outs_pool = ctx.enter_context(tc.tile_pool(name="outs", bufs=3))
        # PSUM: scores get two 2-bank slots (both heads side by side so ONE
        # fused activation exps both); projections/out-proj share two 1-bank
        # slots; ctx accumulators take the last 2 banks (one per head).
        sc_pool = ctx.enter_context(tc.tile_pool(name="sc", bufs=2, space="PSUM"))
        ps_pool = ctx.enter_context(tc.tile_pool(name="ps", bufs=2, space="PSUM"))
        ctx_ps_pool = ctx.enter_context(
            tc.tile_pool(name="ctx_ps", bufs=1, space="PSUM")
        )

        wq_sb = consts.tile([128, DCH, DPC], BF, tag="wq")
        wk_sb = consts.tile([128, DCH, DPC], BF, tag="wk")
        wv_sb = consts.tile([128, DCH, DPC], BF, tag="wv")
        wo_sb = consts.tile([DPC, D], BF, tag="wo")
        bq_sb = consts.tile([DPC, 1], F32, tag="bq")
        for w_sb, w_d in [(wq_sb, wqt), (wk_sb, wkt), (wv_sb, wvt)]:
            nc.sync.dma_start(out=w_sb, in_=w_d[:])
        nc.sync.dma_start(out=wo_sb, in_=woT[:])
        nc.sync.dma_start(out=bq_sb, in_=bq_d[:])

        # K / V stay resident per batch; double-buffered across the two
        # batches so batch 1's projections run during batch 0's chunks.
        KT_sbs = [qk_pool.tile([DPC, S], BF, tag=f"KT{p}", name=f"KT{p}") for p in range(2)]
        v_tiles = [
            [
                v_pool.tile([128, HPC, HD + 1], BF, tag=f"v{p}_{st}", name=f"v{p}_{st}")
                for st in range(NST)
            ]
            for p in range(2)
        ]
        # ones columns (softmax denominator trick) written once; projection
        # rewrites only cols 0:64 so these survive.
        for p in range(2):
            for st in range(NST):
                for h in range(HPC):
                    nc.vector.memset(v_tiles[p][st][:, h, HD : HD + 1], 1.0)

        pending = None  # (ctxT tile, b, chunk) awaiting out-projection

        def emit_out_proj_tt(pend, tt):
            ctxT, ob, och = pend
            out_sb = outs_pool.tile([128, 2, 512], BF, tag="out")
            for eh in range(2):
                po = ps_pool.tile([128, 512], F32, tag="ps", name=f"po{tt}{eh}")
                nc.tensor.matmul(
                    po[:],
                    lhsT=ctxT[:, ts(tt, 128)],
                    rhs=wo_sb[:, ts(eh, 512)],
                    start=True,
                    stop=True,
                )
                if eh == 0:
                    nc.vector.tensor_copy(out=out_sb[:, eh, :], in_=po[:])
                else:
                    nc.scalar.activation(out=out_sb[:, eh, :], in_=po[:], func=COPY)
            t0 = och * 512 + tt * 128
            nc.sync.dma_start(out=outp[ob, t0 : t0 + 128], in_=out_sb)

        def emit_out_proj(pend):
            for tt in range(4):
                emit_out_proj_tt(pend, tt)

        def emit_qproj(qb, qchk):
            qt_sb = ldqk_pool.tile([128, DCH, 512], BF, tag="ldqk")
            nc.sync.dma_start(out=qt_sb, in_=qTt[qb, qchk])
            pq = ps_pool.tile([128, 512], F32, tag="ps", name="pq")
            for c in range(DCH):
                nc.tensor.matmul(
                    pq[:],
                    lhsT=wq_sb[:, c, :],
                    rhs=qt_sb[:, c, :],
                    start=(c == 0),
                    stop=(c == DCH - 1),
                )
            QTc = qtc_pool.tile([DPC, 512], BF, tag="QTc")
            # QT = Q/8 + bq/8 (host sends bq/8; attention scale folded
            # in). On Act so scores never queue behind DVE work.
            nc.scalar.activation(
                out=QTc[:],
                in_=pq[:],
                func=mybir.ActivationFunctionType.Identity,
                bias=bq_sb[:],
                scale=0.125,
            )
            return QTc

        def emit_kgroup(pb, j):
            kt_sb = ldqk_pool.tile([128, DCH, 512], BF, tag="ldqk")
            nc.sync.dma_start(out=kt_sb, in_=kTt[pb, j])
            pk = ps_pool.tile([128, 512], F32, tag="ps", name="pk")
            for c in range(DCH):
                nc.tensor.matmul(
                    pk[:],
                    lhsT=wk_sb[:, c, :],
                    rhs=kt_sb[:, c, :],
                    start=(c == 0),
                    stop=(c == DCH - 1),
                )
            nc.vector.tensor_copy(out=KT_sbs[pb % 2][:, ts(j, 512)], in_=pk[:])

        def emit_vpair(pb, p):
            vt_sb = ldv_pool.tile([128, 2, DCH, 128], BF, tag="ldv")
            nc.sync.dma_start(out=vt_sb, in_=vTt[pb, p])
            for k in range(2):
                pv = ps_pool.tile([128, 512], F32, tag="ps", name="pv")
                for c in range(DCH):
                    nc.tensor.matmul(
                        pv[:, 0:DPC],
                        lhsT=vt_sb[:, k, c, :],
                        rhs=wv_sb[:, c, :],
                        start=(c == 0),
                        stop=(c == DCH - 1),
                    )
                for h in range(HPC):
                    nc.vector.tensor_copy(
                        out=v_tiles[pb % 2][2 * p + k][:, h, 0:HD],
                        in_=pv[:, ts(h, HD)],
                    )

        chunk_seq = [(b, chk) for b in range(B) for chk in range(NCH)]
        # K/V projection schedule: each batch self-JITs inside its first
        # chunk — the K group / V pair task is emitted just before the
        # s-tiles that consume it, so projection DMA+compute overlap the
        # attention pipeline instead of forming a serial lead-in.
        side = {ci: {} for ci in range(len(chunk_seq))}
        for pb in range(B):
            ci0 = pb * NCH
            for j in range(4):
                side[ci0].setdefault(4 * j, []).append((emit_kgroup, pb, j))
            for p in range(8):
                side[ci0].setdefault(2 * p, []).append((emit_vpair, pb, p))

        QTc_next = None
        for ci, (b, chk) in enumerate(chunk_seq):
            if True:
                # Q projection: prefetched at st==8 of the previous chunk so
                # the first scores of this chunk start right at the boundary.
                QTc = QTc_next if QTc_next is not None else emit_qproj(b, chk)
                QTc_next = None
                KT_sb = KT_sbs[b % 2]

                cps = [
                    ctx_ps_pool.tile([128, 512], F32, tag=f"ctx{h}", name=f"cps{h}")
                    for h in range(HPC)
                ]
                for st in range(NST):
                    for task in side[ci].get(st, []):
                        task[0](*task[1:])

                    if st % 2 == 0:
                        bias_sb = bias_pool.tile([128, 2, HPC, 512], BF, tag="bias")
                        nc.sync.dma_start(out=bias_sb, in_=biasTt[b, chk, st // 2])

                    sch = sc_pool.tile([128, HPC, 512], F32, tag="sc", name="sc")
                    sc_mms = []
                    with tc.high_priority(offset=400):
                        for h in range(HPC):
                            mm = nc.tensor.matmul(
                                sch[:, h, :],
                                lhsT=KT_sb[ts(h, HD), ts(st, 128)],
                                rhs=QTc[ts(h, HD), :],
                                start=True,
                                stop=True,
                            )
                            sc_mms.append(mm)
                    add_dep_helper(
                        sc_mms[1].ins, sc_mms[0].ins, sync=False,
                        reason="score pair adjacency",
                    )
                    pt = pt_pool.tile([128, HPC, 512], BF, tag="pt")
                    # one fused exp over both heads' scores (2-bank PSUM read)
                    nc.scalar.activation(out=pt[:], in_=sch[:], func=EXP)
                    # attn_bias enters multiplicatively: host sends exp(bias);
                    # one fused bf16 multiply covers both heads. (GpSimd
                    # measured 2.1us for this op vs DVE 0.65us — keep on DVE.)
                    nc.vector.tensor_tensor(
                        out=pt[:], in0=pt[:], in1=bias_sb[:, st % 2], op=MULT
                    )
                    for h in range(HPC):
                        nc.tensor.matmul(
                            cps[h][0 : HD + 1, :],
                            lhsT=v_tiles[b % 2][st][:, h, :],
                            rhs=pt[:, h, :],
                            start=(st == 0),
                            stop=(st == NST - 1),
                        )
                    if st == 2 and pending is not None:
                        emit_out_proj(pending)
                        pending = None
                    if st == 8 and ci + 1 < len(chunk_seq):
                        QTc_next = emit_qproj(*chunk_seq[ci + 1])

                # ---- chunk boundary: normalize into ctxT ----
                # ctx and its denominator (row 64, from the ones column) sit
                # in one PSUM bank per head; normalization reads PSUM once.
                ctxT = ctxt_pool.tile([DPC, 512], BF, tag="ctxT")
                for h in range(HPC):
                    # reciprocal_approx_fast needs base partition 0
                    den0 = norm_pool.tile([1, 512], F32, tag="den0", name=f"den{h}")
                    nc.scalar.activation(
                        out=den0[:], in_=cps[h][HD : HD + 1, :], func=COPY
                    )
                    rd = norm_pool.tile([1, 512], F32, tag="rd")
                    nc.vector.reciprocal_approx_fast(out=rd[:], in_=den0[:])
                    rrep = norm_pool.tile([128, 512], F32, tag="rrep")
                    nc.gpsimd.partition_broadcast(rrep[:], rd[:])
                    nc.vector.tensor_tensor(
                        out=ctxT[ts(h, HD), :],
                        in0=cps[h][0:HD, :],
                        in1=rrep[0:HD, :],
                        op=MULT,
                    )
                pending = (ctxT, b, chk)

        emit_out_proj(pending)

    nc.compile()
    return nc


def _get_program():
    global _PROGRAM
    if _PROGRAM is None:
        _PROGRAM = build_program()
    return _PROGRAM


def make_in_maps(query, key, value, attn_bias, Wq, bq, Wk, Wv, Wo):
    bf = ml_dtypes.bfloat16
    f32 = np.float32
    query = np.asarray(query, f32)
    key = np.asarray(key, f32)
    value = np.asarray(value, f32)
    attn_bias = np.asarray(attn_bias, f32)
    Wq, Wk, Wv, Wo = (np.asarray(w, f32) for w in (Wq, Wk, Wv, Wo))
    # q/k: [b, ch, p, c, t']; v: [b, st2, p, s2, c, s'] — per-DMA contiguous
    qTt = np.ascontiguousarray(
        query.reshape(B, NCH, 512, DCH, 128).transpose(0, 1, 4, 3, 2)
    ).astype(bf)
    kTt = np.ascontiguousarray(
        key.reshape(B, NCH, 512, DCH, 128).transpose(0, 1, 4, 3, 2)
    ).astype(bf)
    vTt = np.ascontiguousarray(
        value.reshape(B, NST // 2, 2, 128, DCH, 128).transpose(0, 1, 5, 2, 4, 3)
    ).astype(bf)
    in_maps = []
    for cidx in range(NCORES):
        dsl = slice(DPC * cidx, DPC * (cidx + 1))
        hsl = slice(HPC * cidx, HPC * (cidx + 1))
        # [b, h, ch, t', st2, s2, p] -> [b, ch, st2, p, s2, h, t']
        biasTt = np.ascontiguousarray(
            np.exp(attn_bias[:, hsl])
            .reshape(B, HPC, NCH, 512, NST // 2, 2, 128)
            .transpose(0, 2, 4, 6, 5, 1, 3)
        ).astype(bf)
        in_maps.append(
            {
                "qTt": qTt,
                "kTt": kTt,
                "vTt": vTt,
                "biasTt": biasTt,
                "wqt": np.ascontiguousarray(
                    Wq[dsl].T.reshape(DCH, 128, DPC).transpose(1, 0, 2)
                ).astype(bf),
                "wkt": np.ascontiguousarray(
                    Wk[dsl].T.reshape(DCH, 128, DPC).transpose(1, 0, 2)
                ).astype(bf),
                "wvt": np.ascontiguousarray(
                    Wv[dsl].T.reshape(DCH, 128, DPC).transpose(1, 0, 2)
                ).astype(bf),
                "woT": np.ascontiguousarray(Wo[:, dsl].T).astype(bf),
                # bq/8: the Q-projection evacuation computes Q*0.125 + this
                "bq": np.ascontiguousarray(
                    np.asarray(bq, f32)[dsl] * 0.125
                ).reshape(DPC, 1),
            }
        )
    return in_maps


def combine_outputs(results, Wo, bv, bo):
    out = np.zeros((B, T, D), np.float64)
    for c in range(NCORES):
        out += results[c]["outp"].reshape(B, T, D).astype(np.float64)
    const = np.asarray(bv, np.float64) @ np.asarray(Wo, np.float64).T + np.asarray(
        bo, np.float64
    )
    out += const
    return out.astype(np.float32)


def kernel(
    query,
    key,
    value,
    attn_bias,
    key_padding_mask,
    Wq,
    bq,
    Wk,
    bk,
    Wv,
    bv,
    Wo,
    bo,
):
    # key_padding_mask is all-False in this problem; bk is dropped (softmax is
    # invariant to a per-row constant shift); bv/bo enter via a host constant.
    nc = _get_program()
    in_maps = make_in_maps(query, key, value, attn_bias, Wq, bq, Wk, Wv, Wo)
    res = run_bass_kernel_spmd(nc, in_maps, list(range(NCORES)))
    return combine_outputs(res.results, Wo, bv, bo)


if __name__ == "__main__":
    rng = np.random.default_rng(0)
    args = {
        "query": rng.standard_normal((B, T, D), np.float32),
        "key": rng.standard_normal((B, S, D), np.float32),
        "value": rng.standard_normal((B, S, D), np.float32),
        "attn_bias": rng.standard_normal((B, H, T, S), np.float32),
        "key_padding_mask": np.zeros((B, S), bool),
        "Wq": rng.uniform(-0.03125, 0.03125, (D, D)).astype(np.float32),
        "bq": rng.uniform(-0.03125, 0.03125, D).astype(np.float32),
        "Wk": rng.uniform(-0.03125, 0.03125, (D, D)).astype(np.float32),
        "bk": rng.uniform(-0.03125, 0.03125, D).astype(np.float32),
        "Wv": rng.uniform(-0.03125, 0.03125, (D, D)).astype(np.float32),
        "bv": rng.uniform(-0.03125, 0.03125, D).astype(np.float32),
        "Wo": rng.uniform(-0.03125, 0.03125, (D, D)).astype(np.float32),
        "bo": rng.uniform(-0.03125, 0.03125, D).astype(np.float32),
    }
    out = kernel(**args)
    print("kernel ran, out shape", out.shape, "std", out.std())
